# revision 6
# baseline (speedup 1.0000x reference)
"""Trainium2 Bass kernel for the 16-head masked-attention module.

Math per head (reference):
    q = Q @ Wq.T + bq ; k = K @ Wk.T + bk ; v = V @ Wv.T + bv      [S, 64]
    qk = tril(q @ k.T)                 (zeroed, not -inf)
    scores = log_softmax(qk / 8, axis=0)   (softmax over the QUERY axis,
                                            per key column)
    attn = scores @ v
    out = concat(heads) @ WO.T + bO

Device decomposition (8 cores, 2 heads/core, tensor-parallel over heads,
WO row-sharded; host sums the 8 partial outputs, subtracts the per-core
wcorr rows and adds bO):

    scores[s,t] = msc[t,s]/8 - lse[t]   where msc = masked raw qk
    ([t,s] layout), lse[t] = log(t + sum_{s>=t} exp(msc[t,s]/8)).

    attn[s,:] = (1/8)*sum_t msc[t,s] v[t,:]  -  sum_t lse[t] v[t,:]
    out_partial = (1/8)*P(s,:) @ WO_c.T - wcorr   (wcorr is s-independent)

    The 1/8 is folded into wo' = WO_c.T/8 on the host; wcorr =
    (8*corr)^T @ wo' is emitted as a separate [1,1024] output so the WO
    matmuls never wait on the log-sum-exp reduction.

    Fully-kept 128-blocks collapse via rank-64 prefix sums:
        C_m = k_m^T v_m  [64,64];  P_m = sum_{m'<m} C_m'
    Only the 16 diagonal 128x128 triangles are materialized in SBUF.

Dataflow: inputs are streamed panel-major (s-panels of 512, all 8
d-chunks contiguous per panel) in the order K-panel0, Q panels, K
panels 1-3, V panels, so score rows unlock progressively and the exp
grind on ScalarE (the critical engine) starts ~15us in.
"""

import numpy as np

S = 2048
D = 1024
NCORES = 8
PW = 512          # s-panel width
NPAN = 4          # panels per tensor

_CACHE = {}


def _split_multi_waits(nc, mybir, max_waits=1):
    """This walrus build only encodes one sync-wait per instruction; Tile's
    tail drain carries several. Hoist extras onto preceding NoOps."""
    n = 0
    for fn in nc.m.functions:
        for blk in fn.blocks:
            out = []
            changed = False
            for ins in blk.instructions:
                si = getattr(ins, "sync_info", None)
                waits = list(si.on_wait) if (si is not None and si.on_wait) else []
                if len(waits) > max_waits:
                    for w in waits[:-max_waits]:
                        nop = mybir.InstNoOp(
                            name=nc.get_next_instruction_name(), ins=[], outs=[]
                        )
                        nop.engine = ins.engine
                        nop.sync_info = mybir.SyncInfo(on_wait=[w], on_update=[])
                        out.append(nop)
                        n += 1
                    si.on_wait = waits[-max_waits:]
                    changed = True
                out.append(ins)
            if changed:
                blk.instructions = out
    return n


def _build(loop_n=1):
    import concourse.bass as bass
    import concourse.mybir as mybir
    import concourse.tile as tile
    from concourse.bass import ts
    from concourse.masks import make_identity

    F32 = mybir.dt.float32
    BF16 = mybir.dt.bfloat16
    AF = mybir.ActivationFunctionType
    OP = mybir.AluOpType

    nc = bass.Bass("TRN2", num_devices=NCORES, debug=False)

    # panel-major inputs: row = (panel, p128), col = (o8, s512)
    qp_d = nc.dram_tensor("qp", [NPAN * 128, 8 * PW], BF16, kind="ExternalInput")
    kp_d = nc.dram_tensor("kp", [NPAN * 128, 8 * PW], BF16, kind="ExternalInput")
    vp_d = nc.dram_tensor("vp", [NPAN * 128, 8 * PW], BF16, kind="ExternalInput")
    # packed constants: wbf1 = [wq | wk], wbf2 = [wv | wo'] (wo' = WO_c.T/8),
    # cf = [bq | bk | bv | tm | ct] (f32)
    wbf1_d = nc.dram_tensor("wbf1", [128, 2048], BF16, kind="ExternalInput")
    wbf2_d = nc.dram_tensor("wbf2", [128, 2048], BF16, kind="ExternalInput")
    cf_d = nc.dram_tensor("cf", [128, 147], F32, kind="ExternalInput")
    out_d = nc.dram_tensor("out", [S, D], BF16, kind="ExternalOutput")
    wc_d = nc.dram_tensor("wc", [1, D], F32, kind="ExternalOutput")

    with tile.TileContext(nc) as tc:
        with (
            tc.tile_pool(name="singles", bufs=1) as sg,
            tc.tile_pool(name="instream", bufs=3) as instream,
            tc.tile_pool(name="scratch", bufs=2) as scratch,
            tc.tile_pool(name="outs", bufs=4) as outs,
        ):
            wbf1 = sg.tile([128, 2048], BF16, tag="wbf1")
            wbf2 = sg.tile([128, 2048], BF16, tag="wbf2")
            cf = sg.tile([128, 147], F32, tag="cf")
            ident = sg.tile([128, 128], BF16, tag="ident")
            wq_sb = wbf1[:, 0:1024].rearrange("p (o f) -> p o f", f=128)
            wk_sb = wbf1[:, 1024:2048].rearrange("p (o f) -> p o f", f=128)
            wv_sb = wbf2[:, 0:1024].rearrange("p (o f) -> p o f", f=128)
            wo_sb = wbf2[:, 1024:2048]
            bq_sb = cf[:, 0:1]
            bk_sb = cf[:, 1:2]
            bv_sb = cf[:, 2:3]
            tm_sb = cf[:, 3:131]
            ct_sb = cf[:, 131:147]

            # persistent activations
            qT = sg.tile([128, S], BF16, tag="qT")   # [dk(2 heads), s]
            kT = sg.tile([128, S], BF16, tag="kT")
            vT = sg.tile([128, S], BF16, tag="vT")
            k_sb = sg.tile([128, 16, 128], BF16, tag="k_sb")  # [t, chunk, dk]
            v_sb = sg.tile([128, 16, 128], BF16, tag="v_sb")  # [t, chunk, dv]
            p_f32 = sg.tile([128, 16, 64], F32, tag="p_f32")  # prefix sums
            p_bf = sg.tile([128, 16, 64], BF16, tag="p_bf")
            zT = sg.tile([128, S], BF16, tag="zT")
            sums_m = [sg.tile([128, 16], F32, tag=f"s_m{h}", name=f"s_m{h}")
                      for h in range(2)]
            sums_t = [sg.tile([128, 4], F32, tag=f"s_t{h}", name=f"s_t{h}")
                      for h in range(2)]
            lse_f = [sg.tile([128, 16], F32, tag=f"lse_f{h}", name=f"lse_f{h}")
                     for h in range(2)]
            lse8 = [sg.tile([128, 16], BF16, tag=f"lse8{h}", name=f"lse8{h}")
                    for h in range(2)]
            corr_bf = sg.tile([128, 1], BF16, tag="corr")
            wc_sb = sg.tile([1, D], F32, tag="wc_sb")
            tri = [
                [sg.tile([128, 128], BF16, tag=f"tri{h}_{i}", name=f"tri{h}_{i}")
                 for i in range(16)]
                for h in range(2)
            ]

            def emit_body():
                _emit_phases(
                    nc, tc, tile, mybir, ts, F32, BF16, AF, OP,
                    qp_d, kp_d, vp_d, wbf2_d, out_d, wc_d,
                    wbf1, wbf2, cf, make_identity,
                    wq_sb, wk_sb, wv_sb, wo_sb, bq_sb, bk_sb, bv_sb,
                    tm_sb, ct_sb, ident, instream, scratch, outs,
                    qT, kT, vT, k_sb, v_sb, p_f32, p_bf, zT,
                    sums_m, sums_t, lse_f, lse8, corr_bf, wc_sb, tri,
                    wbf1_d, cf_d,
                )

            if loop_n == 1:
                emit_body()
            else:
                with tc.For_i(0, loop_n, 1):
                    emit_body()

    _split_multi_waits(nc, mybir)
    return nc


def _emit_phases(
    nc, tc, tile, mybir, ts, F32, BF16, AF, OP,
    qp_d, kp_d, vp_d, wbf2_d, out_d, wc_d,
    wbf1, wbf2, cf, make_identity,
    wq_sb, wk_sb, wv_sb, wo_sb, bq_sb, bk_sb, bv_sb,
    tm_sb, ct_sb, ident, instream, scratch, outs,
    qT, kT, vT, k_sb, v_sb, p_f32, p_bf, zT,
    sums_m, sums_t, lse_f, lse8, corr_bf, wc_sb, tri,
    wbf1_d, cf_d,
):
    nc.sync.dma_start(wbf1[:], wbf1_d[:])
    nc.sync.dma_start(cf[:], cf_d[:])
    make_identity(nc, ident[:])

    # pool open order fixes bank assignment: pp and psct get the low banks
    # (freed early), psc the high ones (live until the last exp) — phase 4
    # then reuses pp/psct's banks and can run inside the exp window.
    with tc.tile_pool(name="pp", bufs=1, space="PSUM") as pp, \
         tc.tile_pool(name="psct", bufs=1, space="PSUM") as psct, \
         tc.tile_pool(name="psc", bufs=2, space="PSUM") as psc:

        def proj_panel(src_d, p, w_sb, b_sb, dstT, name, on_act):
            """One s-panel (512 cols) of a [dk,s] projection."""
            pan = instream.tile([128, 4096], BF16, tag="pan", name=f"pan_{name}{p}")
            nc.sync.dma_start(pan[:], src_d[ts(p, 128), :])
            ps = pp.tile([128, PW], F32, tag="pp", name=f"pp_{name}{p}")
            for o in range(8):
                nc.tensor.matmul(
                    ps[:], w_sb[:, o, :], pan[:, ts(o, PW)],
                    start=(o == 0), stop=(o == 7),
                )
            if on_act:
                nc.scalar.activation(
                    dstT[:, ts(p, PW)], ps[:], AF.Identity,
                    bias=b_sb[:], scale=1.0,
                )
            else:
                nc.vector.tensor_scalar(
                    dstT[:, ts(p, PW)], ps[:], b_sb[:], None, op0=OP.add
                )

        def score_row(i, h):
            """Score row chunk i (128 keys), head h: matmuls + diag mask +
            exp sweep(s) with accumulated per-row sums."""
            j0 = i // 4
            width = min((4 - j0) * 512, 1536)
            hp = slice(64 * h, 64 * h + 64)
            big = psc.tile([128, 1536], F32, tag="psc", name=f"sc_{h}_{i}")
            for jj in range(width // 512):
                nc.tensor.matmul(
                    big[:, ts(jj, 512)],
                    kT[hp, ts(i, 128)],
                    qT[hp, ts(j0 + jj, 512)],
                    start=True, stop=True,
                )
            dcol = 128 * (i % 4)
            nc.vector.tensor_tensor(
                tri[h][i][:], big[:, dcol:dcol + 128], tm_sb[:], OP.mult
            )
            nc.vector.tensor_copy(big[:, dcol:dcol + 128], tri[h][i][:])
            e1 = scratch.tile([128, 1536], BF16, tag="exp1", name=f"e1_{h}_{i}")
            nc.scalar.activation(
                e1[:, : width - dcol], big[:, dcol:width], AF.Exp,
                scale=0.125, accum_out=sums_m[h][:, i:i + 1],
            )
            if j0 == 0:
                # tail block, cols 1536..2048, separate accumulator slot
                tl = psct.tile([128, 512], F32, tag="psct", name=f"sct_{h}_{i}")
                nc.tensor.matmul(
                    tl[:], kT[hp, ts(i, 128)], qT[hp, ts(3, 512)],
                    start=True, stop=True,
                )
                e2 = scratch.tile([128, 512], BF16, tag="exp2",
                                  name=f"e2_{h}_{i}")
                nc.scalar.activation(
                    e2[:], tl[:], AF.Exp,
                    scale=0.125, accum_out=sums_t[h][:, i:i + 1],
                )

        # ---- K panel 0 first (unlocks score rows 0-3 right after Q) ----
        proj_panel(kp_d, 0, wk_sb, bk_sb, kT, "k", on_act=True)
        for p in range(NPAN):
            proj_panel(qp_d, p, wq_sb, bq_sb, qT, "q", on_act=True)
        for i in range(4):
            for h in range(2):
                score_row(i, h)
        for p in range(1, NPAN):
            proj_panel(kp_d, p, wk_sb, bk_sb, kT, "k", on_act=False)
            for i in range(4 * p, 4 * p + 4):
                for h in range(2):
                    score_row(i, h)

        # ---- V stream + vT + k_sb/v_sb transposes + prefix C/P ----
        nc.sync.dma_start(wbf2[:], wbf2_d[:])
        nc.vector.memset(p_f32[:, 0, :], 0.0)
        nc.vector.tensor_copy(p_bf[:, 0, :], p_f32[:, 0, :])
        for p in range(NPAN):
            proj_panel(vp_d, p, wv_sb, bv_sb, vT, "v", on_act=False)
            for m in range(4 * p, 4 * p + 4):
                ptk = pp.tile([128, 128], BF16, tag="pp", name=f"ptk_{m}")
                nc.tensor.transpose(ptk[:], kT[:, ts(m, 128)], ident[:])
                nc.vector.tensor_copy(k_sb[:, m, :], ptk[:])
                ptv = pp.tile([128, 128], BF16, tag="pp", name=f"ptv_{m}")
                nc.tensor.transpose(ptv[:], vT[:, ts(m, 128)], ident[:])
                nc.vector.tensor_copy(v_sb[:, m, :], ptv[:])
                if m < 15:
                    ctile = pp.tile([128, 64], F32, tag="pp", name=f"pc_{m}")
                    nc.tensor.matmul(
                        ctile[0:64, :], k_sb[:, m, 0:64], v_sb[:, m, 0:64],
                        start=True, stop=True,
                    )
                    nc.tensor.matmul(
                        ctile[64:128, :], k_sb[:, m, 64:128], v_sb[:, m, 64:128],
                        start=True, stop=True, tile_position=(0, 64),
                    )
                    nc.vector.tensor_tensor(
                        p_f32[:, m + 1, :], p_f32[:, m, :], ctile[:], OP.add
                    )
                    nc.vector.tensor_copy(p_bf[:, m + 1, :], p_f32[:, m + 1, :])

    # ---- Phase 4: attention assembly + WO (no corr dependency) ----
    with tc.tile_pool(name="p4", bufs=2, space="PSUM") as p4:
        for g in range(4):
            patt = p4.tile([128, 512], F32, tag="p4", name=f"pat_{g}")
            for mm in range(4):
                m = 4 * g + mm
                cols = ts(mm, 128)
                if m > 0:
                    nc.tensor.matmul(
                        patt[0:64, cols], p_bf[0:64, m, :],
                        qT[0:64, ts(m, 128)], start=True, stop=False,
                    )
                    nc.tensor.matmul(
                        patt[64:128, cols], p_bf[64:128, m, :],
                        qT[64:128, ts(m, 128)],
                        start=True, stop=False, tile_position=(64, 64),
                    )
                nc.tensor.matmul(
                    patt[0:64, cols], v_sb[:, m, 0:64], tri[0][m][:],
                    start=(m == 0), stop=True,
                )
                nc.tensor.matmul(
                    patt[64:128, cols], v_sb[:, m, 64:128], tri[1][m][:],
                    start=(m == 0), stop=True, tile_position=(0, 64),
                )
            nc.vector.tensor_copy(zT[:, ts(g, 512)], patt[:])
            for mm in range(4):
                m = 4 * g + mm
                o_sb = outs.tile([128, D], BF16, tag="osb", name=f"osb_{m}")
                for half in range(2):
                    cols = slice(512 * half, 512 * half + 512)
                    po = p4.tile([128, 512], F32, tag="p4",
                                 name=f"pwo_{m}_{half}")
                    nc.tensor.matmul(
                        po[:], zT[:, ts(m, 128)], wo_sb[:, cols],
                        start=True, stop=True,
                    )
                    nc.vector.tensor_copy(o_sb[:, cols], po[:])
                nc.sync.dma_start(out_d[ts(m, 128), :], o_sb[:])

    # ---- corr tail: lse -> corr8 -> wcorr [1, 1024] ----
    with tc.tile_pool(name="pcr", bufs=1, space="PSUM") as pcr, \
         tc.tile_pool(name="pwc", bufs=2, space="PSUM") as pwc:
        for h in range(2):
            nc.vector.tensor_tensor(
                lse_f[h][:], sums_m[h][:], ct_sb[:], OP.add
            )
            nc.vector.tensor_tensor(
                lse_f[h][:, 0:4], lse_f[h][:, 0:4], sums_t[h][:], OP.add
            )
            nc.scalar.activation(lse_f[h][:], lse_f[h][:], AF.Ln, scale=1.0)
            # lse8 = 8*lse (folds the 1/8 score scale removed from WO)
            nc.scalar.activation(lse8[h][:], lse_f[h][:], AF.Identity, scale=8.0)
        cps = pcr.tile([128, 1], F32, tag="pcr")
        for i in range(16):
            nc.tensor.matmul(
                cps[0:64, :], v_sb[:, i, 0:64], lse8[0][:, i:i + 1],
                start=(i == 0), stop=(i == 15),
            )
            nc.tensor.matmul(
                cps[64:128, :], v_sb[:, i, 64:128], lse8[1][:, i:i + 1],
                start=(i == 0), stop=(i == 15), tile_position=(0, 64),
            )
        nc.vector.tensor_copy(corr_bf[:], cps[:])
        for half in range(2):
            cols = slice(512 * half, 512 * half + 512)
            wps = pwc.tile([1, 512], F32, tag="pwc", name=f"pwc_{half}")
            nc.tensor.matmul(
                wps[:], corr_bf[:], wo_sb[:, cols], start=True, stop=True,
            )
            nc.vector.tensor_copy(wc_sb[:, cols], wps[:])
        nc.sync.dma_start(wc_d[:], wc_sb[:])


def _get_program(loop_n=1):
    key = f"nc{loop_n}"
    if key not in _CACHE:
        _CACHE[key] = _build(loop_n)
    return _CACHE[key]


def _get_exec(loop_n=1):
    """Build the sharded PJRT executable once (same lowering path as
    concourse.bass2jax.run_bass_via_pjrt, hoisted so repeat calls don't
    re-trace/re-compile)."""
    key = f"exec{loop_n}"
    if key in _CACHE:
        return _CACHE[key]
    import jax
    import numpy as _np
    from jax.experimental.shard_map import shard_map
    from jax.sharding import Mesh, PartitionSpec
    import concourse.mybir as mybir
    from concourse import bass2jax

    nc = _get_program(loop_n)
    bass2jax.install_neuronx_cc_hook()

    partition_name = (
        nc.partition_id_tensor.name if nc.partition_id_tensor else None
    )
    in_names, out_names, out_avals = [], [], []
    for alloc in nc.m.functions[0].allocations:
        if not isinstance(alloc, mybir.MemoryLocationSet):
            continue
        name = alloc.memorylocations[0].name
        if alloc.kind == "ExternalInput":
            if name != partition_name:
                in_names.append(name)
        elif alloc.kind == "ExternalOutput":
            out_names.append(name)
            out_avals.append(
                jax.core.ShapedArray(
                    tuple(alloc.tensor_shape), mybir.dt.np(alloc.dtype)
                )
            )
    n_params = len(in_names)
    n_outs = len(out_avals)
    all_names = in_names + out_names
    if partition_name is not None:
        all_names = all_names + [partition_name]

    def _body(*args):
        operands = list(args)
        if partition_name is not None:
            operands.append(bass2jax.partition_id_tensor())
        outs = bass2jax._bass_exec_p.bind(
            *operands,
            out_avals=tuple(out_avals),
            in_names=tuple(all_names),
            out_names=tuple(out_names),
            lowering_input_output_aliases=(),
            sim_require_finite=True,
            sim_require_nnan=True,
            nc=nc,
        )
        return tuple(outs)

    devices = jax.devices()[:NCORES]
    mesh = Mesh(_np.asarray(devices), ("core",))
    donate = tuple(range(n_params, n_params + n_outs))
    sharded = jax.jit(
        shard_map(
            _body,
            mesh=mesh,
            in_specs=(PartitionSpec("core"),) * (n_params + n_outs),
            out_specs=(PartitionSpec("core"),) * n_outs,
            check_rep=False,
        ),
        donate_argnums=donate,
        keep_unused=True,
    )
    _CACHE[key] = (sharded, in_names, out_names, out_avals, mesh)
    return _CACHE[key]


def _run(in_maps, loop_n=1):
    """Execute on 8 cores; returns list of per-core output dicts."""
    import numpy as _np

    sharded, in_names, out_names, out_avals, mesh = _get_exec(loop_n)
    concat_in = [
        _np.concatenate([m[name] for m in in_maps], axis=0) for name in in_names
    ]
    concat_zeros = [
        _np.zeros((NCORES * a.shape[0], *a.shape[1:]), a.dtype) for a in out_avals
    ]
    out_arrs = sharded(*concat_in, *concat_zeros)
    return [
        {
            name: _np.asarray(out_arrs[i]).reshape(NCORES, *out_avals[i].shape)[c]
            for i, name in enumerate(out_names)
        }
        for c in range(NCORES)
    ]


def bench(in_maps, iters=5, loop_n=1):
    """Time device execution with device-resident inputs (excludes host
    transfer of the big operands; zero output buffers are pre-staged)."""
    import time

    import jax
    import numpy as _np
    from jax.sharding import NamedSharding, PartitionSpec

    sharded, in_names, out_names, out_avals, mesh = _get_exec(loop_n)
    sh = NamedSharding(mesh, PartitionSpec("core"))
    concat_in = [
        jax.device_put(
            _np.concatenate([m[name] for m in in_maps], axis=0), sh
        )
        for name in in_names
    ]
    zeros_pool = [
        [
            jax.device_put(
                _np.zeros((NCORES * a.shape[0], *a.shape[1:]), a.dtype), sh
            )
            for a in out_avals
        ]
        for _ in range(iters + 1)
    ]
    for z in zeros_pool:
        for a in z:
            a.block_until_ready()
    # warm-up
    outs = sharded(*concat_in, *zeros_pool[0])
    jax.block_until_ready(outs)
    times = []
    for it in range(iters):
        t0 = time.perf_counter()
        outs = sharded(*concat_in, *zeros_pool[it + 1])
        jax.block_until_ready(outs)
        times.append(time.perf_counter() - t0)
    return times, outs


def _panel_major(x):
    """[S, D] f32 -> [(pan p), (o s)] bf16 with
    arr[pan, p, o, s] = x[pan*PW + s, o*128 + p]."""
    import ml_dtypes

    a = np.asarray(x, np.float32).reshape(NPAN, PW, 8, 128)
    a = np.ascontiguousarray(a.transpose(0, 3, 2, 1))
    return a.reshape(NPAN * 128, 8 * PW).astype(ml_dtypes.bfloat16)


def kernel(Q_input, K_input, V_input, WQw, WQb, WKw, WKb, WVw, WVb, WOw, WOb,
           _return_results=False):
    import ml_dtypes

    BF = ml_dtypes.bfloat16

    qp = _panel_major(Q_input)
    kp = _panel_major(K_input)
    vp = _panel_major(V_input)

    # triangular keep-mask M[p, c] = 1 if c >= p, and per-chunk skip counts
    tm = (np.arange(128)[None, :] >= np.arange(128)[:, None]).astype(np.float32)
    ct = np.broadcast_to(
        (128.0 * np.arange(16, dtype=np.float32))[None, :], (128, 16)
    ).copy()

    in_maps = []
    for c in range(NCORES):
        h0 = 2 * c

        def _prep_w(w):
            # [2, 64, D] -> [D, 128] -> partition-major [128, 8, 128]
            wt = np.asarray(w, np.float32).transpose(2, 0, 1).reshape(D, 128)
            return np.ascontiguousarray(
                wt.reshape(8, 128, 128).transpose(1, 0, 2)
            ).astype(BF)

        wq = _prep_w(WQw[h0:h0 + 2])
        wk = _prep_w(WKw[h0:h0 + 2])
        wv = _prep_w(WVw[h0:h0 + 2])
        # wo' = (1/8) * WO_c^T  (the 1/8 score scale folded in)
        wo = np.ascontiguousarray(
            np.asarray(WOw, np.float32)[:, 128 * c:128 * (c + 1)].T * 0.125
        ).astype(BF)
        wbf1 = np.concatenate(
            [wq.reshape(128, 1024), wk.reshape(128, 1024)], axis=1
        )
        wbf2 = np.concatenate([wv.reshape(128, 1024), wo], axis=1)
        cf = np.concatenate(
            [
                np.asarray(WQb[h0:h0 + 2], np.float32).reshape(128, 1),
                np.asarray(WKb[h0:h0 + 2], np.float32).reshape(128, 1),
                np.asarray(WVb[h0:h0 + 2], np.float32).reshape(128, 1),
                tm, ct,
            ],
            axis=1,
        )
        in_maps.append({
            "qp": qp, "kp": kp, "vp": vp,
            "wbf1": wbf1, "wbf2": wbf2, "cf": np.ascontiguousarray(cf),
        })

    results = _run(in_maps)
    out = np.zeros((S, D), np.float64)
    for c in range(NCORES):
        out += results[c]["out"].astype(np.float64)
        out -= results[c]["wc"].astype(np.float64)
    out += np.asarray(WOb, np.float32)[None, :]
    if _return_results:
        return out.astype(np.float32), (results, in_maps)
    return out.astype(np.float32)


# revision 11
# speedup vs baseline: 1.0007x; 1.0007x over previous
"""Trainium2 Bass kernel for the 16-head masked-attention module.

Math per head (reference):
    q = Q @ Wq.T + bq ; k = K @ Wk.T + bk ; v = V @ Wv.T + bv      [S, 64]
    qk = tril(q @ k.T)                 (zeroed, not -inf)
    scores = log_softmax(qk / 8, axis=0)   (softmax over the QUERY axis,
                                            per key column)
    attn = scores @ v
    out = concat(heads) @ WO.T + bO

Device decomposition (8 cores, 2 heads/core, tensor-parallel over heads,
WO row-sharded; host sums the 8 partial outputs, subtracts the per-core
wcorr rows and adds bO):

    scores[s,t] = msc[t,s]/8 - lse[t]   where msc = masked raw qk
    ([t,s] layout), lse[t] = log(t + sum_{s>=t} exp(msc[t,s]/8)).

    attn[s,:] = (1/8)*sum_t msc[t,s] v[t,:]  -  sum_t lse[t] v[t,:]
    out_partial = (1/8)*P(s,:) @ WO_c.T - wcorr   (wcorr is s-independent)

    The 1/8 is folded into wo' = WO_c.T/8 on the host; wcorr =
    (8*corr)^T @ wo' is emitted as a separate [1,1024] output so the WO
    matmuls never wait on the log-sum-exp reduction.

    Fully-kept 128-blocks collapse via rank-64 prefix sums:
        C_m = k_m^T v_m  [64,64];  P_m = sum_{m'<m} C_m'
    Only the 16 diagonal 128x128 triangles are materialized in SBUF.

Dataflow: inputs are streamed panel-major (s-panels of 512, all 8
d-chunks contiguous per panel) in the order K-panel0, Q panels, K
panels 1-3, V panels, so score rows unlock progressively and the exp
grind on ScalarE (the critical engine) starts ~15us in.
"""

import numpy as np

S = 2048
D = 1024
NCORES = 8
PW = 512          # s-panel width
NPAN = 4          # panels per tensor

_CACHE = {}


def _split_multi_waits(nc, mybir, max_waits=1):
    """This walrus build only encodes one sync-wait per instruction; Tile's
    tail drain carries several. Hoist extras onto preceding NoOps."""
    n = 0
    for fn in nc.m.functions:
        for blk in fn.blocks:
            out = []
            changed = False
            for ins in blk.instructions:
                si = getattr(ins, "sync_info", None)
                waits = list(si.on_wait) if (si is not None and si.on_wait) else []
                if len(waits) > max_waits:
                    for w in waits[:-max_waits]:
                        nop = mybir.InstNoOp(
                            name=nc.get_next_instruction_name(), ins=[], outs=[]
                        )
                        nop.engine = ins.engine
                        nop.sync_info = mybir.SyncInfo(on_wait=[w], on_update=[])
                        out.append(nop)
                        n += 1
                    si.on_wait = waits[-max_waits:]
                    changed = True
                out.append(ins)
            if changed:
                blk.instructions = out
    return n


def _build(loop_n=1):
    import concourse.bass as bass
    import concourse.mybir as mybir
    import concourse.tile as tile
    from concourse.bass import ts
    from concourse.masks import make_identity

    F32 = mybir.dt.float32
    BF16 = mybir.dt.bfloat16
    AF = mybir.ActivationFunctionType
    OP = mybir.AluOpType

    nc = bass.Bass("TRN2", num_devices=NCORES, debug=False)

    # panel-major inputs: row = (panel, p128), col = (o8, s512)
    qp_d = nc.dram_tensor("qp", [NPAN * 128, 8 * PW], BF16, kind="ExternalInput")
    kp_d = nc.dram_tensor("kp", [NPAN * 128, 8 * PW], BF16, kind="ExternalInput")
    vp_d = nc.dram_tensor("vp", [NPAN * 128, 8 * PW], BF16, kind="ExternalInput")
    # packed constants: wbf1 = [wq | wk], wbf2 = [wv | wo'] (wo' = WO_c.T/8),
    # cf = [bq | bk | bv | tm | ct] (f32)
    wbf1_d = nc.dram_tensor("wbf1", [128, 2048], BF16, kind="ExternalInput")
    wbf2_d = nc.dram_tensor("wbf2", [128, 2048], BF16, kind="ExternalInput")
    cf_d = nc.dram_tensor("cf", [128, 147], F32, kind="ExternalInput")
    out_d = nc.dram_tensor("out", [S, D], BF16, kind="ExternalOutput")
    wc_d = nc.dram_tensor("wc", [1, D], F32, kind="ExternalOutput")

    with tile.TileContext(nc) as tc:
        with (
            tc.tile_pool(name="singles", bufs=1) as sg,
            tc.tile_pool(name="instream", bufs=3) as instream,
            tc.tile_pool(name="scratch", bufs=2) as scratch,
            tc.tile_pool(name="outs", bufs=4) as outs,
        ):
            wbf1 = sg.tile([128, 2048], BF16, tag="wbf1")
            wbf2 = sg.tile([128, 2048], BF16, tag="wbf2")
            cf = sg.tile([128, 147], F32, tag="cf")
            ident = sg.tile([128, 128], BF16, tag="ident")
            wq_sb = wbf1[:, 0:1024].rearrange("p (o f) -> p o f", f=128)
            wk_sb = wbf1[:, 1024:2048].rearrange("p (o f) -> p o f", f=128)
            wv_sb = wbf2[:, 0:1024].rearrange("p (o f) -> p o f", f=128)
            wo_sb = wbf2[:, 1024:2048]
            bq_sb = cf[:, 0:1]
            bk_sb = cf[:, 1:2]
            bv_sb = cf[:, 2:3]
            tm_sb = cf[:, 3:131]
            ct_sb = cf[:, 131:147]

            # persistent activations
            qT = sg.tile([128, S], BF16, tag="qT")   # [dk(2 heads), s]
            kT = sg.tile([128, S], BF16, tag="kT")
            vT = sg.tile([128, S], BF16, tag="vT")
            k_sb = sg.tile([128, 16, 128], BF16, tag="k_sb")  # [t, chunk, dk]
            v_sb = sg.tile([128, 16, 128], BF16, tag="v_sb")  # [t, chunk, dv]
            p_f32 = sg.tile([128, 16, 64], F32, tag="p_f32")  # prefix sums
            p_bf = sg.tile([128, 16, 64], BF16, tag="p_bf")
            zT = sg.tile([128, S], BF16, tag="zT")
            sums_m = [sg.tile([128, 16], F32, tag=f"s_m{h}", name=f"s_m{h}")
                      for h in range(2)]
            sums_t = [sg.tile([128, 8], F32, tag=f"s_t{h}", name=f"s_t{h}")
                      for h in range(2)]
            lse_f = [sg.tile([128, 16], F32, tag=f"lse_f{h}", name=f"lse_f{h}")
                     for h in range(2)]
            lse8 = [sg.tile([128, 16], BF16, tag=f"lse8{h}", name=f"lse8{h}")
                    for h in range(2)]
            corr_bf = sg.tile([128, 1], BF16, tag="corr")
            wc_sb = sg.tile([1, D], F32, tag="wc_sb")
            tri = [
                [sg.tile([128, 128], BF16, tag=f"tri{h}_{i}", name=f"tri{h}_{i}")
                 for i in range(16)]
                for h in range(2)
            ]

            def emit_body():
                _emit_phases(
                    nc, tc, tile, mybir, ts, F32, BF16, AF, OP,
                    qp_d, kp_d, vp_d, wbf2_d, out_d, wc_d,
                    wbf1, wbf2, cf, make_identity,
                    wq_sb, wk_sb, wv_sb, wo_sb, bq_sb, bk_sb, bv_sb,
                    tm_sb, ct_sb, ident, instream, scratch, outs,
                    qT, kT, vT, k_sb, v_sb, p_f32, p_bf, zT,
                    sums_m, sums_t, lse_f, lse8, corr_bf, wc_sb, tri,
                    wbf1_d, cf_d,
                )

            if loop_n == 1:
                emit_body()
            else:
                with tc.For_i(0, loop_n, 1):
                    emit_body()

    _split_multi_waits(nc, mybir)
    return nc


def _emit_phases(
    nc, tc, tile, mybir, ts, F32, BF16, AF, OP,
    qp_d, kp_d, vp_d, wbf2_d, out_d, wc_d,
    wbf1, wbf2, cf, make_identity,
    wq_sb, wk_sb, wv_sb, wo_sb, bq_sb, bk_sb, bv_sb,
    tm_sb, ct_sb, ident, instream, scratch, outs,
    qT, kT, vT, k_sb, v_sb, p_f32, p_bf, zT,
    sums_m, sums_t, lse_f, lse8, corr_bf, wc_sb, tri,
    wbf1_d, cf_d,
):
    nc.sync.dma_start(wbf1[:], wbf1_d[:])
    nc.sync.dma_start(cf[:], cf_d[:])
    make_identity(nc, ident[:])

    # PSUM budget (8 banks): pp 1 + psct 2 + psc 2x2 + p4 1 = 8. p4 holds a
    # dedicated bank for phase 4/WO so the scheduler can run it inside the
    # exp window instead of after the last score row frees a bank.
    with tc.tile_pool(name="pp", bufs=1, space="PSUM") as pp, \
         tc.tile_pool(name="psct", bufs=1, space="PSUM") as psct, \
         tc.tile_pool(name="psc", bufs=2, space="PSUM") as psc, \
         tc.tile_pool(name="p4", bufs=1, space="PSUM") as p4:

        def proj_panel(src_d, p, w_sb, b_sb, dstT, name, on_act):
            """One s-panel (512 cols) of a [dk,s] projection."""
            pan = instream.tile([128, 4096], BF16, tag="pan", name=f"pan_{name}{p}")
            nc.sync.dma_start(pan[:], src_d[ts(p, 128), :])
            ps = pp.tile([128, PW], F32, tag="pp", name=f"pp_{name}{p}")
            for o in range(8):
                nc.tensor.matmul(
                    ps[:], w_sb[:, o, :], pan[:, ts(o, PW)],
                    start=(o == 0), stop=(o == 7),
                )
            if on_act:
                nc.scalar.activation(
                    dstT[:, ts(p, PW)], ps[:], AF.Identity,
                    bias=b_sb[:], scale=1.0,
                )
            else:
                nc.vector.tensor_scalar(
                    dstT[:, ts(p, PW)], ps[:], b_sb[:], None, op0=OP.add
                )

        def score_row(i, h):
            """Score row chunk i (128 keys), head h: matmuls + diag mask +
            exp sweep(s) with accumulated per-row sums."""
            j0 = i // 4
            width = min(1024, 2048 - 512 * j0)  # main tile width
            hp = slice(64 * h, 64 * h + 64)
            big = psc.tile([128, 1024], F32, tag="psc", name=f"sc_{h}_{i}")
            for jj in range(width // 512):
                nc.tensor.matmul(
                    big[:, ts(jj, 512)],
                    kT[hp, ts(i, 128)],
                    qT[hp, ts(j0 + jj, 512)],
                    start=True, stop=True,
                )
            dcol = 128 * (i % 4)
            nc.vector.tensor_tensor(
                tri[h][i][:], big[:, dcol:dcol + 128], tm_sb[:], OP.mult
            )
            nc.vector.tensor_copy(big[:, dcol:dcol + 128], tri[h][i][:])
            e1 = scratch.tile([128, 1024], BF16, tag="exp1", name=f"e1_{h}_{i}")
            nc.scalar.activation(
                e1[:, : width - dcol], big[:, dcol:width], AF.Exp,
                scale=0.125, accum_out=sums_m[h][:, i:i + 1],
            )
            twidth = (2 - j0) * 512  # tail: cols 512*j0+1024 .. 2048
            if twidth > 0:
                tl = psct.tile([128, 1024], F32, tag="psct", name=f"sct_{h}_{i}")
                for jj in range(twidth // 512):
                    nc.tensor.matmul(
                        tl[:, ts(jj, 512)],
                        kT[hp, ts(i, 128)],
                        qT[hp, ts(j0 + 2 + jj, 512)],
                        start=True, stop=True,
                    )
                e2 = scratch.tile([128, 1024], BF16, tag="exp2",
                                  name=f"e2_{h}_{i}")
                nc.scalar.activation(
                    e2[:, :twidth], tl[:, :twidth], AF.Exp,
                    scale=0.125, accum_out=sums_t[h][:, i:i + 1],
                )

        # ---- K panel 0 first (unlocks score rows 0-3 right after Q) ----
        proj_panel(kp_d, 0, wk_sb, bk_sb, kT, "k", on_act=True)
        for p in range(NPAN):
            proj_panel(qp_d, p, wq_sb, bq_sb, qT, "q", on_act=True)
        for i in range(4):
            for h in range(2):
                score_row(i, h)
        for p in range(1, NPAN):
            proj_panel(kp_d, p, wk_sb, bk_sb, kT, "k", on_act=False)
            for i in range(4 * p, 4 * p + 4):
                for h in range(2):
                    score_row(i, h)

        # ---- V stream + vT (full speed), then transposes + prefix C/P ----
        nc.sync.dma_start(wbf2[:], wbf2_d[:])
        nc.vector.memset(p_f32[:, 0, :], 0.0)
        nc.vector.tensor_copy(p_bf[:, 0, :], p_f32[:, 0, :])
        for p in range(NPAN):
            proj_panel(vp_d, p, wv_sb, bv_sb, vT, "v", on_act=False)
        for m in range(16):
            ptk = pp.tile([128, 128], BF16, tag="pp", name=f"ptk_{m}")
            nc.tensor.transpose(ptk[:], kT[:, ts(m, 128)], ident[:])
            nc.vector.tensor_copy(k_sb[:, m, :], ptk[:])
            ptv = pp.tile([128, 128], BF16, tag="pp", name=f"ptv_{m}")
            nc.tensor.transpose(ptv[:], vT[:, ts(m, 128)], ident[:])
            nc.vector.tensor_copy(v_sb[:, m, :], ptv[:])
            if m < 15:
                ctile = pp.tile([128, 64], F32, tag="pp", name=f"pc_{m}")
                nc.tensor.matmul(
                    ctile[0:64, :], k_sb[:, m, 0:64], v_sb[:, m, 0:64],
                    start=True, stop=True,
                )
                nc.tensor.matmul(
                    ctile[64:128, :], k_sb[:, m, 64:128], v_sb[:, m, 64:128],
                    start=True, stop=True, tile_position=(0, 64),
                )
                nc.vector.tensor_tensor(
                    p_f32[:, m + 1, :], p_f32[:, m, :], ctile[:], OP.add
                )
                nc.vector.tensor_copy(p_bf[:, m + 1, :], p_f32[:, m + 1, :])

        # ---- Phase 4: attention assembly + WO (no corr dependency) ----
        for g in range(4):
            patt = p4.tile([128, 512], F32, tag="p4", name=f"pat_{g}")
            for mm in range(4):
                m = 4 * g + mm
                cols = ts(mm, 128)
                if m > 0:
                    nc.tensor.matmul(
                        patt[0:64, cols], p_bf[0:64, m, :],
                        qT[0:64, ts(m, 128)], start=True, stop=False,
                    )
                    nc.tensor.matmul(
                        patt[64:128, cols], p_bf[64:128, m, :],
                        qT[64:128, ts(m, 128)],
                        start=True, stop=False, tile_position=(64, 64),
                    )
                nc.tensor.matmul(
                    patt[0:64, cols], v_sb[:, m, 0:64], tri[0][m][:],
                    start=(m == 0), stop=True,
                )
                nc.tensor.matmul(
                    patt[64:128, cols], v_sb[:, m, 64:128], tri[1][m][:],
                    start=(m == 0), stop=True, tile_position=(0, 64),
                )
            nc.vector.tensor_copy(zT[:, ts(g, 512)], patt[:])
            for mm in range(4):
                m = 4 * g + mm
                o_sb = outs.tile([128, D], BF16, tag="osb", name=f"osb_{m}")
                for half in range(2):
                    cols = slice(512 * half, 512 * half + 512)
                    po = p4.tile([128, 512], F32, tag="p4",
                                 name=f"pwo_{m}_{half}")
                    nc.tensor.matmul(
                        po[:], zT[:, ts(m, 128)], wo_sb[:, cols],
                        start=True, stop=True,
                    )
                    nc.vector.tensor_copy(o_sb[:, cols], po[:])
                nc.sync.dma_start(out_d[ts(m, 128), :], o_sb[:])

    # ---- corr tail: lse -> corr8 -> wcorr [1, 1024] ----
    with tc.tile_pool(name="pcr", bufs=1, space="PSUM") as pcr, \
         tc.tile_pool(name="pwc", bufs=2, space="PSUM") as pwc:
        for h in range(2):
            nc.vector.tensor_tensor(
                lse_f[h][:], sums_m[h][:], ct_sb[:], OP.add
            )
            nc.vector.tensor_tensor(
                lse_f[h][:, 0:8], lse_f[h][:, 0:8], sums_t[h][:], OP.add
            )
            nc.scalar.activation(lse_f[h][:], lse_f[h][:], AF.Ln, scale=1.0)
            # lse8 = 8*lse (folds the 1/8 score scale removed from WO)
            nc.scalar.activation(lse8[h][:], lse_f[h][:], AF.Identity, scale=8.0)
        cps = pcr.tile([128, 1], F32, tag="pcr")
        for i in range(16):
            nc.tensor.matmul(
                cps[0:64, :], v_sb[:, i, 0:64], lse8[0][:, i:i + 1],
                start=(i == 0), stop=(i == 15),
            )
            nc.tensor.matmul(
                cps[64:128, :], v_sb[:, i, 64:128], lse8[1][:, i:i + 1],
                start=(i == 0), stop=(i == 15), tile_position=(0, 64),
            )
        nc.vector.tensor_copy(corr_bf[:], cps[:])
        for half in range(2):
            cols = slice(512 * half, 512 * half + 512)
            wps = pwc.tile([1, 512], F32, tag="pwc", name=f"pwc_{half}")
            nc.tensor.matmul(
                wps[:], corr_bf[:], wo_sb[:, cols], start=True, stop=True,
            )
            nc.vector.tensor_copy(wc_sb[:, cols], wps[:])
        nc.sync.dma_start(wc_d[:], wc_sb[:])


def _get_program(loop_n=1):
    key = f"nc{loop_n}"
    if key not in _CACHE:
        _CACHE[key] = _build(loop_n)
    return _CACHE[key]


def _get_exec(loop_n=1):
    """Build the sharded PJRT executable once (same lowering path as
    concourse.bass2jax.run_bass_via_pjrt, hoisted so repeat calls don't
    re-trace/re-compile)."""
    key = f"exec{loop_n}"
    if key in _CACHE:
        return _CACHE[key]
    import jax
    import numpy as _np
    from jax.experimental.shard_map import shard_map
    from jax.sharding import Mesh, PartitionSpec
    import concourse.mybir as mybir
    from concourse import bass2jax

    nc = _get_program(loop_n)
    bass2jax.install_neuronx_cc_hook()

    partition_name = (
        nc.partition_id_tensor.name if nc.partition_id_tensor else None
    )
    in_names, out_names, out_avals = [], [], []
    for alloc in nc.m.functions[0].allocations:
        if not isinstance(alloc, mybir.MemoryLocationSet):
            continue
        name = alloc.memorylocations[0].name
        if alloc.kind == "ExternalInput":
            if name != partition_name:
                in_names.append(name)
        elif alloc.kind == "ExternalOutput":
            out_names.append(name)
            out_avals.append(
                jax.core.ShapedArray(
                    tuple(alloc.tensor_shape), mybir.dt.np(alloc.dtype)
                )
            )
    n_params = len(in_names)
    n_outs = len(out_avals)
    all_names = in_names + out_names
    if partition_name is not None:
        all_names = all_names + [partition_name]

    def _body(*args):
        operands = list(args)
        if partition_name is not None:
            operands.append(bass2jax.partition_id_tensor())
        outs = bass2jax._bass_exec_p.bind(
            *operands,
            out_avals=tuple(out_avals),
            in_names=tuple(all_names),
            out_names=tuple(out_names),
            lowering_input_output_aliases=(),
            sim_require_finite=True,
            sim_require_nnan=True,
            nc=nc,
        )
        return tuple(outs)

    devices = jax.devices()[:NCORES]
    mesh = Mesh(_np.asarray(devices), ("core",))
    donate = tuple(range(n_params, n_params + n_outs))
    sharded = jax.jit(
        shard_map(
            _body,
            mesh=mesh,
            in_specs=(PartitionSpec("core"),) * (n_params + n_outs),
            out_specs=(PartitionSpec("core"),) * n_outs,
            check_rep=False,
        ),
        donate_argnums=donate,
        keep_unused=True,
    )
    _CACHE[key] = (sharded, in_names, out_names, out_avals, mesh)
    return _CACHE[key]


def _run(in_maps, loop_n=1):
    """Execute on 8 cores; returns list of per-core output dicts."""
    import numpy as _np

    sharded, in_names, out_names, out_avals, mesh = _get_exec(loop_n)
    concat_in = [
        _np.concatenate([m[name] for m in in_maps], axis=0) for name in in_names
    ]
    concat_zeros = [
        _np.zeros((NCORES * a.shape[0], *a.shape[1:]), a.dtype) for a in out_avals
    ]
    out_arrs = sharded(*concat_in, *concat_zeros)
    return [
        {
            name: _np.asarray(out_arrs[i]).reshape(NCORES, *out_avals[i].shape)[c]
            for i, name in enumerate(out_names)
        }
        for c in range(NCORES)
    ]


def bench(in_maps, iters=5, loop_n=1):
    """Time device execution with device-resident inputs (excludes host
    transfer of the big operands; zero output buffers are pre-staged)."""
    import time

    import jax
    import numpy as _np
    from jax.sharding import NamedSharding, PartitionSpec

    sharded, in_names, out_names, out_avals, mesh = _get_exec(loop_n)
    sh = NamedSharding(mesh, PartitionSpec("core"))
    concat_in = [
        jax.device_put(
            _np.concatenate([m[name] for m in in_maps], axis=0), sh
        )
        for name in in_names
    ]
    zeros_pool = [
        [
            jax.device_put(
                _np.zeros((NCORES * a.shape[0], *a.shape[1:]), a.dtype), sh
            )
            for a in out_avals
        ]
        for _ in range(iters + 1)
    ]
    for z in zeros_pool:
        for a in z:
            a.block_until_ready()
    # warm-up
    outs = sharded(*concat_in, *zeros_pool[0])
    jax.block_until_ready(outs)
    times = []
    for it in range(iters):
        t0 = time.perf_counter()
        outs = sharded(*concat_in, *zeros_pool[it + 1])
        jax.block_until_ready(outs)
        times.append(time.perf_counter() - t0)
    return times, outs


def _panel_major(x):
    """[S, D] f32 -> [(pan p), (o s)] bf16 with
    arr[pan, p, o, s] = x[pan*PW + s, o*128 + p]."""
    import ml_dtypes

    a = np.asarray(x, np.float32).reshape(NPAN, PW, 8, 128)
    a = np.ascontiguousarray(a.transpose(0, 3, 2, 1))
    return a.reshape(NPAN * 128, 8 * PW).astype(ml_dtypes.bfloat16)


def kernel(Q_input, K_input, V_input, WQw, WQb, WKw, WKb, WVw, WVb, WOw, WOb,
           _return_results=False):
    import ml_dtypes

    BF = ml_dtypes.bfloat16

    qp = _panel_major(Q_input)
    kp = _panel_major(K_input)
    vp = _panel_major(V_input)

    # triangular keep-mask M[p, c] = 1 if c >= p, and per-chunk skip counts
    tm = (np.arange(128)[None, :] >= np.arange(128)[:, None]).astype(np.float32)
    ct = np.broadcast_to(
        (128.0 * np.arange(16, dtype=np.float32))[None, :], (128, 16)
    ).copy()

    in_maps = []
    for c in range(NCORES):
        h0 = 2 * c

        def _prep_w(w):
            # [2, 64, D] -> [D, 128] -> partition-major [128, 8, 128]
            wt = np.asarray(w, np.float32).transpose(2, 0, 1).reshape(D, 128)
            return np.ascontiguousarray(
                wt.reshape(8, 128, 128).transpose(1, 0, 2)
            ).astype(BF)

        wq = _prep_w(WQw[h0:h0 + 2])
        wk = _prep_w(WKw[h0:h0 + 2])
        wv = _prep_w(WVw[h0:h0 + 2])
        # wo' = (1/8) * WO_c^T  (the 1/8 score scale folded in)
        wo = np.ascontiguousarray(
            np.asarray(WOw, np.float32)[:, 128 * c:128 * (c + 1)].T * 0.125
        ).astype(BF)
        wbf1 = np.concatenate(
            [wq.reshape(128, 1024), wk.reshape(128, 1024)], axis=1
        )
        wbf2 = np.concatenate([wv.reshape(128, 1024), wo], axis=1)
        cf = np.concatenate(
            [
                np.asarray(WQb[h0:h0 + 2], np.float32).reshape(128, 1),
                np.asarray(WKb[h0:h0 + 2], np.float32).reshape(128, 1),
                np.asarray(WVb[h0:h0 + 2], np.float32).reshape(128, 1),
                tm, ct,
            ],
            axis=1,
        )
        in_maps.append({
            "qp": qp, "kp": kp, "vp": vp,
            "wbf1": wbf1, "wbf2": wbf2, "cf": np.ascontiguousarray(cf),
        })

    results = _run(in_maps)
    out = np.zeros((S, D), np.float64)
    for c in range(NCORES):
        out += results[c]["out"].astype(np.float64)
        out -= results[c]["wc"].astype(np.float64)
    out += np.asarray(WOb, np.float32)[None, :]
    if _return_results:
        return out.astype(np.float32), (results, in_maps)
    return out.astype(np.float32)
